# revision 1
# baseline (speedup 1.0000x reference)
"""MoE down-projection + topk-weighted combine (moe_reduce_rs) on 8 NeuronCores.

Strategy (tensor-parallel over tokens x hidden, zero collectives, zero
indirect DMA):

- Grid: 2 token-halves x 4 H-slices. Core (h, j) handles 4096 tokens and a
  512-wide slice of H. Each core holds its own [8, 512, 512] weight slice
  resident in SBUF (8 MB).
- Host groups tokens by their ordered expert pair (e0, e1) -> 64 groups per
  half, balanced across the two halves so the static per-group capacity C is
  ceil(max_pair_count / 2). Rows are pre-scaled by the topk weights; tokens
  with e0 == e1 merge both slots into one row (single matmul chain).
- Device: per pair group one PSUM tile [C, 512] accumulates 8 float32r
  matmuls (2 slots x 4 K-chunks; 4 for diagonal groups). Output rows are
  written contiguously in group order; the host applies the inverse
  permutation (pure data movement) when assembling the full output.

float32r runs the PE at 1 cycle/row (vs 4 for float32) with ~1e-3 max
relative error at K=512 - measured on hardware.
"""

import math
import sys
import types

import numpy as np

NTOK = 8192
TOPK = 2
E, I, H = 8, 512, 2048
NG = 2        # token halves
NJ = 4        # H slices
HSL = H // NJ # 512
KC = I // 128 # 4 contraction chunks
N_CORES = 8


def _install_ntff_hook():
    """bass_utils' axon trace path imports antenv.axon_hooks, which this
    image lacks; provide it via the boot shim so BASS_TRACE=1 works."""
    if "antenv.axon_hooks" in sys.modules:
        return
    try:
        from trn_agent_boot.trn_boot import _ntff_profile_via_ctypes
        hook = _ntff_profile_via_ctypes("/opt/axon/libaxon_pjrt.so")
        mod = types.ModuleType("antenv.axon_hooks")
        mod.get_axon_ntff_profile_hook = lambda: hook
        sys.modules["antenv.axon_hooks"] = mod
    except Exception:
        pass


_install_ntff_hook()

_NC_CACHE = {}
LAST_RESULTS = None  # BassKernelResults of the most recent run (for profiling)


def _build_nc(C):
    import concourse.tile as tile
    import concourse.mybir as mybir
    from concourse import bacc

    RT = E * E * C
    NT = math.ceil(C / 128)  # row sub-tiles per pair group
    f32 = mybir.dt.float32
    f32r = mybir.dt.float32r

    nc = bacc.Bacc(None, target_bir_lowering=False)
    xa_d = nc.declare_dram_parameter("xa", [I, RT], f32r, isOutput=False)
    xb_d = nc.declare_dram_parameter("xb", [I, RT], f32r, isOutput=False)
    w_d = nc.declare_dram_parameter("w", [E, I, HSL], f32r, isOutput=False)
    y_d = nc.declare_dram_parameter("y", [RT, HSL], f32, isOutput=True)

    xa_r = xa_d.rearrange("(c p) r -> p c r", p=128)  # [128, KC, RT]
    xb_r = xb_d.rearrange("(c p) r -> p c r", p=128)

    with tile.TileContext(nc) as tc:
        with (
            tc.tile_pool(name="wpool", bufs=1) as wpool,
            tc.tile_pool(name="xpool", bufs=4) as xpool,
            tc.tile_pool(name="opool", bufs=4) as opool,
            tc.tile_pool(name="psum", bufs=8, space="PSUM") as psum_pool,
        ):
            w_t = []
            for e in range(E):
                wt = wpool.tile([128, KC, HSL], f32r, name=f"w{e}")
                nc.sync.dma_start(wt[:], w_d[e].rearrange("(c p) n -> p c n", p=128))
                w_t.append(wt)

            for p in range(E * E):
                a, b = p // E, p % E
                diag = a == b
                xa_t = xpool.tile([128, KC, C], f32r, name="xa_t")
                nc.sync.dma_start(xa_t[:], xa_r[:, :, p * C:(p + 1) * C])
                if not diag:
                    xb_t = xpool.tile([128, KC, C], f32r, name="xb_t")
                    nc.sync.dma_start(xb_t[:], xb_r[:, :, p * C:(p + 1) * C])
                for s in range(NT):
                    m0 = s * 128
                    M = min(128, C - m0)
                    acc = psum_pool.tile([M, HSL], f32, name="acc")
                    for c in range(KC):
                        nc.tensor.matmul(
                            acc[:], xa_t[:, c, m0:m0 + M], w_t[a][:, c, :],
                            start=(c == 0), stop=(diag and c == KC - 1))
                    if not diag:
                        for c in range(KC):
                            nc.tensor.matmul(
                                acc[:], xb_t[:, c, m0:m0 + M], w_t[b][:, c, :],
                                start=False, stop=(c == KC - 1))
                    out_t = opool.tile([M, HSL], f32, name="out_t")
                    nc.vector.tensor_copy(out_t[:], acc[:])
                    nc.sync.dma_start(y_d[p * C + m0:p * C + m0 + M, :], out_t[:])
    nc.compile()
    return nc


def kernel(intermediate_states, down_weight, full_topk_ids, full_topk_weight):
    from concourse.bass_utils import run_bass_kernel_spmd
    global LAST_RESULTS

    x = np.ascontiguousarray(np.asarray(intermediate_states, dtype=np.float32))
    W = np.ascontiguousarray(np.asarray(down_weight, dtype=np.float32))
    ids = np.asarray(full_topk_ids).astype(np.int64)
    tw = np.asarray(full_topk_weight, dtype=np.float32)

    ntok = ids.shape[0]
    # scaled slot rows
    xs0 = x[0::2] * tw[:, 0:1]   # [ntok, I]
    xs1 = x[1::2] * tw[:, 1:2]

    pairv = ids[:, 0] * E + ids[:, 1]            # [ntok]
    order = np.argsort(pairv, kind="stable")
    sorted_pair = pairv[order]
    cnt = np.bincount(pairv, minlength=E * E)
    starts = np.zeros(E * E, np.int64)
    starts[1:] = np.cumsum(cnt)[:-1]
    posin = np.arange(ntok, dtype=np.int64) - starts[sorted_pair]
    halfsel = (posin & 1).astype(np.int64)       # balanced split across halves
    rank = posin >> 1

    C = int(math.ceil(cnt.max() / 2))
    C = (C + 7) // 8 * 8
    RT = E * E * C

    diag_tok = ids[:, 0] == ids[:, 1]            # [ntok]

    xaT = np.zeros((NG, I, RT), dtype=np.float32)
    xbT = np.zeros((NG, I, RT), dtype=np.float32)
    assemble = []  # (tokens, rows) per half
    for h in range(NG):
        m = halfsel == h
        toks = order[m]
        rows = (sorted_pair[m] * C + rank[m]).astype(np.int64)
        dg = diag_tok[toks]
        xa_rows = np.zeros((RT, I), dtype=np.float32)
        xa_rows[rows] = xs0[toks]
        xa_rows[rows[dg]] += xs1[toks[dg]]
        xb_rows = np.zeros((RT, I), dtype=np.float32)
        xb_rows[rows[~dg]] = xs1[toks[~dg]]
        xaT[h] = xa_rows.T
        xbT[h] = xb_rows.T
        assemble.append((toks, rows))

    wsl = [np.ascontiguousarray(W[:, :, j * HSL:(j + 1) * HSL]) for j in range(NJ)]

    if C not in _NC_CACHE:
        _NC_CACHE[C] = _build_nc(C)
    nc = _NC_CACHE[C]

    in_maps = []
    for core in range(N_CORES):
        h, j = core // NJ, core % NJ
        in_maps.append({
            "xa": np.ascontiguousarray(xaT[h]),
            "xb": np.ascontiguousarray(xbT[h]),
            "w": wsl[j],
        })

    res = run_bass_kernel_spmd(nc, in_maps, list(range(N_CORES)))
    LAST_RESULTS = res

    y = np.empty((ntok, H), dtype=np.float32)
    for core in range(N_CORES):
        h, j = core // NJ, core % NJ
        toks, rows = assemble[h]
        y[toks, j * HSL:(j + 1) * HSL] = res.results[core]["y"][rows, :]
    return y


# revision 12
# speedup vs baseline: 1.8762x; 1.8762x over previous
"""MoE down-projection + topk-weighted combine (moe_reduce_rs) on 8 NeuronCores.

Strategy (tensor-parallel over tokens x hidden, zero collectives, zero
indirect DMA):

- Grid: 2 token-halves x 4 H-slices. Core (h, j) handles 4096 tokens and a
  512-wide slice of H. Each core holds its own [8, 512, 512] weight slice
  resident in SBUF (8 MB).
- Host groups tokens by their ordered expert pair (e0, e1) -> 64 groups per
  half, balanced across the two halves so the static per-group capacity C is
  ceil(max_pair_count / 2). Rows are pre-scaled by the topk weights; tokens
  with e0 == e1 merge both slots into one row (single matmul chain).
- Device: per pair group one PSUM tile [C, 512] accumulates 8 float32r
  matmuls (2 slots x 4 K-chunks; 4 for diagonal groups). Output rows are
  written contiguously in group order; the host applies the inverse
  permutation (pure data movement) when assembling the full output.

float32r runs the PE at 1 cycle/row (vs 4 for float32) with ~1e-3 max
relative error at K=512 - measured on hardware.
"""

import math
import os
import sys
import types

import numpy as np

# matmul operand dtype: float16 halves input DMA vs float32r at ~2x the
# (still tiny) rounding error; both run the PE at 1 cycle/row.
MM_DTYPE = os.environ.get("KERNEL_MM_DTYPE", "float16")

NTOK = 8192
TOPK = 2
E, I, H = 8, 512, 2048
NG = 2        # token halves
NJ = 4        # H slices
HSL = H // NJ # 512
KC = I // 128 # 4 contraction chunks
N_CORES = 8


def _install_ntff_hook():
    """bass_utils' axon trace path imports antenv.axon_hooks, which this
    image lacks; provide it via the boot shim so BASS_TRACE=1 works."""
    if "antenv.axon_hooks" in sys.modules:
        return
    try:
        from trn_agent_boot.trn_boot import _ntff_profile_via_ctypes
        hook = _ntff_profile_via_ctypes("/opt/axon/libaxon_pjrt.so")
        mod = types.ModuleType("antenv.axon_hooks")
        mod.get_axon_ntff_profile_hook = lambda: hook
        sys.modules["antenv.axon_hooks"] = mod
    except Exception:
        pass


_install_ntff_hook()

_NC_CACHE = {}
LAST_RESULTS = None  # BassKernelResults of the most recent run (for profiling)


def _build_nc(C):
    import concourse.tile as tile
    import concourse.mybir as mybir
    from concourse import bacc

    RT = E * E * C
    NT = math.ceil(C / 128)  # row sub-tiles per pair group
    f32 = mybir.dt.float32
    f32r = getattr(mybir.dt, MM_DTYPE)

    nc = bacc.Bacc(None, target_bir_lowering=False)
    # xa and xb interleaved per pair: cols [2pC, 2pC+C) = slot-A rows,
    # [2pC+C, 2pC+2C) = slot-B rows -> one load DMA per pair
    xab_d = nc.declare_dram_parameter("xab", [I, 2 * RT], f32r, isOutput=False)
    w_d = nc.declare_dram_parameter("w", [E, I, HSL], f32r, isOutput=False)
    y_d = nc.declare_dram_parameter("y", [RT, HSL], f32, isOutput=True)

    xab_r = xab_d.rearrange("(c p) r -> p c r", p=128)  # [128, KC, 2*RT]

    # Wavefront pair order: shell k introduces expert k, so W_e loads
    # overlap with compute on earlier shells instead of serializing at start.
    pair_order = []
    for k in range(E):
        pair_order.append(k * E + k)                    # (k, k) first
        for a in range(k):
            pair_order.append(a * E + k)                # (a, k)
            pair_order.append(k * E + a)                # (k, a)

    with tile.TileContext(nc) as tc:
        with (
            tc.tile_pool(name="wpool", bufs=1) as wpool,
            tc.tile_pool(name="xpool", bufs=8) as xpool,
            tc.tile_pool(name="opool", bufs=10) as opool,
            tc.tile_pool(name="psum", bufs=8, space="PSUM") as psum_pool,
        ):
            w_t = [wpool.tile([128, KC, HSL], f32r, name=f"w{e}") for e in range(E)]
            w_src = [w_d[e].rearrange("(c p) n -> p c n", p=128) for e in range(E)]
            # W streams in 640 KB per-kchunk quanta interleaved with the pair
            # loads on the SP ring: small quanta never stall a pair's xa/xb
            # load for long, and the pump schedule keeps chunk arrival ahead
            # of the wavefront's first use of each expert.
            # Experts 0-3 ride the ACT ring ahead of any store (FIFO puts
            # them first), so the SP ring starts with nothing but pair
            # loads; experts 4-7 trickle on the SP ring one chunk per pair
            # (their first use is far behind the wavefront).
            for e in range(E // 2):
                for c in range(KC):
                    nc.scalar.dma_start(w_t[e][:, c, :], w_src[e][:, c, :])
            w_jobs = [(e, c) for e in range(E // 2, E) for c in range(KC)]
            w_pos = 0

            def pump_w(n):
                nonlocal w_pos
                for _ in range(n):
                    if w_pos < len(w_jobs):
                        e, c = w_jobs[w_pos]
                        nc.sync.dma_start(w_t[e][:, c, :], w_src[e][:, c, :])
                        w_pos += 1

            for pi, p in enumerate(pair_order):
                a, b = p // E, p % E
                diag = a == b
                xw = C if diag else 2 * C
                xab_t = xpool.tile([128, KC, 2 * C], f32r, name="xab_t")
                nc.sync.dma_start(xab_t[:, :, :xw],
                                  xab_r[:, :, 2 * p * C:2 * p * C + xw])
                pump_w(1)
                for s in range(NT):
                    m0 = s * 128
                    M = min(128, C - m0)
                    acc = psum_pool.tile([M, HSL], f32, name="acc")
                    for c in range(KC):
                        nc.tensor.matmul(
                            acc[:], xab_t[:, c, m0:m0 + M], w_t[a][:, c, :],
                            start=(c == 0), stop=(diag and c == KC - 1))
                    if not diag:
                        for c in range(KC):
                            nc.tensor.matmul(
                                acc[:], xab_t[:, c, C + m0:C + m0 + M], w_t[b][:, c, :],
                                start=False, stop=(c == KC - 1))
                    out_t = opool.tile([M, HSL], f32, name="out_t")
                    nc.vector.tensor_copy(out_t[:], acc[:])
                    # stores go on the ACT HWDGE ring so they can't stall
                    # xa/xb/W loads on the SP ring
                    nc.scalar.dma_start(y_d[p * C + m0:p * C + m0 + M, :], out_t[:])
    nc.compile()
    return nc


def kernel(intermediate_states, down_weight, full_topk_ids, full_topk_weight):
    from concourse.bass_utils import run_bass_kernel_spmd
    global LAST_RESULTS

    x = np.ascontiguousarray(np.asarray(intermediate_states, dtype=np.float32))
    W = np.ascontiguousarray(np.asarray(down_weight, dtype=np.float32))
    ids = np.asarray(full_topk_ids).astype(np.int64)
    tw = np.asarray(full_topk_weight, dtype=np.float32)

    ntok = ids.shape[0]
    # scaled slot rows
    xs0 = x[0::2] * tw[:, 0:1]   # [ntok, I]
    xs1 = x[1::2] * tw[:, 1:2]

    pairv = ids[:, 0] * E + ids[:, 1]            # [ntok]
    order = np.argsort(pairv, kind="stable")
    sorted_pair = pairv[order]
    cnt = np.bincount(pairv, minlength=E * E)
    starts = np.zeros(E * E, np.int64)
    starts[1:] = np.cumsum(cnt)[:-1]
    posin = np.arange(ntok, dtype=np.int64) - starts[sorted_pair]
    halfsel = (posin & 1).astype(np.int64)       # balanced split across halves
    rank = posin >> 1

    C = int(math.ceil(cnt.max() / 2))
    C = (C + 7) // 8 * 8
    RT = E * E * C

    diag_tok = ids[:, 0] == ids[:, 1]            # [ntok]

    np_mm0 = np.float16 if MM_DTYPE == "float16" else np.float32
    xabT = np.zeros((NG, I, 2 * RT), dtype=np_mm0)
    assemble = []  # (tokens, rows) per half
    for h in range(NG):
        m = halfsel == h
        toks = order[m]
        rows = (sorted_pair[m] * C + rank[m]).astype(np.int64)
        dg = diag_tok[toks]
        xab_rows = np.zeros((2 * RT, I), dtype=np.float32)
        arows = rows // C * (2 * C) + rows % C          # slot-A position
        xab_rows[arows] = xs0[toks]
        xab_rows[arows[dg]] += xs1[toks[dg]]
        xab_rows[arows[~dg] + C] = xs1[toks[~dg]]       # slot-B position
        xabT[h] = xab_rows.T.astype(np_mm0)
        assemble.append((toks, rows))

    np_mm = np_mm0
    wsl = [np.ascontiguousarray(W[:, :, j * HSL:(j + 1) * HSL]).astype(np_mm)
           for j in range(NJ)]

    key = (C, MM_DTYPE)
    if key not in _NC_CACHE:
        _NC_CACHE[key] = _build_nc(C)
    nc = _NC_CACHE[key]

    in_maps = []
    for core in range(N_CORES):
        h, j = core // NJ, core % NJ
        in_maps.append({
            "xab": np.ascontiguousarray(xabT[h]),
            "w": wsl[j],
        })

    res = run_bass_kernel_spmd(nc, in_maps, list(range(N_CORES)))
    LAST_RESULTS = res

    y = np.empty((ntok, H), dtype=np.float32)
    for core in range(N_CORES):
        h, j = core // NJ, core % NJ
        toks, rows = assemble[h]
        y[toks, j * HSL:(j + 1) * HSL] = res.results[core]["y"][rows, :]
    return y


# revision 18
# speedup vs baseline: 1.9688x; 1.0493x over previous
"""MoE down-projection + topk-weighted combine (moe_reduce_rs) on 8 NeuronCores.

Strategy (tensor-parallel over tokens x hidden, zero collectives, zero
indirect DMA):

- Grid: 2 token-halves x 4 H-slices. Core (h, j) handles 4096 tokens and a
  512-wide slice of H. Each core holds its own [8, 512, 512] weight slice
  resident in SBUF (8 MB).
- Host groups tokens by their ordered expert pair (e0, e1) -> 64 groups per
  half, balanced across the two halves so the static per-group capacity C is
  ceil(max_pair_count / 2). Rows are pre-scaled by the topk weights; tokens
  with e0 == e1 merge both slots into one row (single matmul chain).
- Device: per pair group one PSUM tile [C, 512] accumulates 8 matmuls
  (2 slots x 4 K-chunks; 4 for diagonal groups, fp32 accumulation in PSUM).
  Output rows are written contiguously in group order; the host applies the
  inverse permutation (pure data movement) when assembling the full output.
- Loads (xab, one DMA per pair) ride the SP HWDGE ring; W (resident, 8
  tiles) and y stores ride the ACT ring so a blocked store can never stall
  a latency-critical load in the ring FIFO.

Operands are float16 (PE at 1 cycle/row like float32r, but FWL weight loads
and half the DMA bytes); PSUM accumulates fp32. Measured end-to-end max
relative error vs the fp32 reference: ~3e-4 (float32r: ~1.3e-4, selectable
via KERNEL_MM_DTYPE=float32r; plain fp32 runs the PE 4x slower).
"""

import math
import os
import sys
import types

import numpy as np

# matmul operand dtype: float16 halves input DMA vs float32r at ~2x the
# (still tiny) rounding error; both run the PE at 1 cycle/row.
MM_DTYPE = os.environ.get("KERNEL_MM_DTYPE", "float16")

NTOK = 8192
TOPK = 2
E, I, H = 8, 512, 2048
NG = 2        # token halves
NJ = 4        # H slices
HSL = H // NJ # 512
KC = I // 128 # 4 contraction chunks
N_CORES = 8


def _install_ntff_hook():
    """bass_utils' axon trace path imports antenv.axon_hooks, which this
    image lacks; provide it via the boot shim so BASS_TRACE=1 works."""
    if "antenv.axon_hooks" in sys.modules:
        return
    try:
        from trn_agent_boot.trn_boot import _ntff_profile_via_ctypes
        hook = _ntff_profile_via_ctypes("/opt/axon/libaxon_pjrt.so")
        mod = types.ModuleType("antenv.axon_hooks")
        mod.get_axon_ntff_profile_hook = lambda: hook
        sys.modules["antenv.axon_hooks"] = mod
    except Exception:
        pass


_install_ntff_hook()

_NC_CACHE = {}
LAST_RESULTS = None  # BassKernelResults of the most recent run (for profiling)


def _build_nc(C):
    import concourse.tile as tile
    import concourse.mybir as mybir
    from concourse import bacc

    RT = E * E * C
    NT = math.ceil(C / 128)  # row sub-tiles per pair group
    f32 = mybir.dt.float32
    f32r = getattr(mybir.dt, MM_DTYPE)

    nc = bacc.Bacc(None, target_bir_lowering=False)
    # xa and xb interleaved per pair: cols [2pC, 2pC+C) = slot-A rows,
    # [2pC+C, 2pC+2C) = slot-B rows -> one load DMA per pair
    xab_d = nc.declare_dram_parameter("xab", [I, 2 * RT], f32r, isOutput=False)
    w_d = nc.declare_dram_parameter("w", [E, I, HSL], f32r, isOutput=False)
    y_d = nc.declare_dram_parameter("y", [RT, HSL], f32, isOutput=True)

    xab_r = xab_d.rearrange("(c p) r -> p c r", p=128)  # [128, KC, 2*RT]

    # Wavefront pair order: shell k introduces expert k, so W_e loads
    # overlap with compute on earlier shells instead of serializing at start.
    pair_order = []
    for k in range(E):
        pair_order.append(k * E + k)                    # (k, k) first
        for a in range(k):
            pair_order.append(a * E + k)                # (a, k)
            pair_order.append(k * E + a)                # (k, a)

    with tile.TileContext(nc) as tc:
        with (
            tc.tile_pool(name="wpool", bufs=1) as wpool,
            tc.tile_pool(name="xpool", bufs=8) as xpool,
            tc.tile_pool(name="opool", bufs=10) as opool,
            tc.tile_pool(name="psum", bufs=8, space="PSUM") as psum_pool,
        ):
            w_t = [wpool.tile([128, KC, HSL], f32r, name=f"w{e}") for e in range(E)]
            w_src = [w_d[e].rearrange("(c p) n -> p c n", p=128) for e in range(E)]
            # All W chunks are issued on the ACT ring before any store, so
            # the FIFO delivers them first while the SP ring carries nothing
            # but pair loads; the wavefront pair order means expert k's
            # first use trails its load by several shells.
            for e in range(E):
                for c in range(KC):
                    nc.scalar.dma_start(w_t[e][:, c, :], w_src[e][:, c, :])

            for pi, p in enumerate(pair_order):
                a, b = p // E, p % E
                diag = a == b
                xw = C if diag else 2 * C
                xab_t = xpool.tile([128, KC, 2 * C], f32r, name="xab_t")
                nc.sync.dma_start(xab_t[:, :, :xw],
                                  xab_r[:, :, 2 * p * C:2 * p * C + xw])

                for s in range(NT):
                    m0 = s * 128
                    M = min(128, C - m0)
                    acc = psum_pool.tile([M, HSL], f32, name="acc")
                    for c in range(KC):
                        nc.tensor.matmul(
                            acc[:], xab_t[:, c, m0:m0 + M], w_t[a][:, c, :],
                            start=(c == 0), stop=(diag and c == KC - 1))
                    if not diag:
                        for c in range(KC):
                            nc.tensor.matmul(
                                acc[:], xab_t[:, c, C + m0:C + m0 + M], w_t[b][:, c, :],
                                start=False, stop=(c == KC - 1))
                    out_t = opool.tile([M, HSL], f32, name="out_t")
                    nc.vector.tensor_copy(out_t[:], acc[:])
                    # stores go on the ACT HWDGE ring so they can't stall
                    # xa/xb/W loads on the SP ring
                    nc.scalar.dma_start(y_d[p * C + m0:p * C + m0 + M, :], out_t[:])
    nc.compile()
    return nc


def kernel(intermediate_states, down_weight, full_topk_ids, full_topk_weight):
    from concourse.bass_utils import run_bass_kernel_spmd
    global LAST_RESULTS

    x = np.ascontiguousarray(np.asarray(intermediate_states, dtype=np.float32))
    W = np.ascontiguousarray(np.asarray(down_weight, dtype=np.float32))
    ids = np.asarray(full_topk_ids).astype(np.int64)
    tw = np.asarray(full_topk_weight, dtype=np.float32)

    ntok = ids.shape[0]
    # scaled slot rows
    xs0 = x[0::2] * tw[:, 0:1]   # [ntok, I]
    xs1 = x[1::2] * tw[:, 1:2]

    pairv = ids[:, 0] * E + ids[:, 1]            # [ntok]
    order = np.argsort(pairv, kind="stable")
    sorted_pair = pairv[order]
    cnt = np.bincount(pairv, minlength=E * E)
    starts = np.zeros(E * E, np.int64)
    starts[1:] = np.cumsum(cnt)[:-1]
    posin = np.arange(ntok, dtype=np.int64) - starts[sorted_pair]
    halfsel = (posin & 1).astype(np.int64)       # balanced split across halves
    rank = posin >> 1

    C = int(math.ceil(cnt.max() / 2))
    C = (C + 7) // 8 * 8
    RT = E * E * C

    diag_tok = ids[:, 0] == ids[:, 1]            # [ntok]

    np_mm0 = np.float16 if MM_DTYPE == "float16" else np.float32
    xabT = np.zeros((NG, I, 2 * RT), dtype=np_mm0)
    assemble = []  # (tokens, rows) per half
    for h in range(NG):
        m = halfsel == h
        toks = order[m]
        rows = (sorted_pair[m] * C + rank[m]).astype(np.int64)
        dg = diag_tok[toks]
        xab_rows = np.zeros((2 * RT, I), dtype=np.float32)
        arows = rows // C * (2 * C) + rows % C          # slot-A position
        xab_rows[arows] = xs0[toks]
        xab_rows[arows[dg]] += xs1[toks[dg]]
        xab_rows[arows[~dg] + C] = xs1[toks[~dg]]       # slot-B position
        xabT[h] = xab_rows.T.astype(np_mm0)
        assemble.append((toks, rows))

    np_mm = np_mm0
    wsl = [np.ascontiguousarray(W[:, :, j * HSL:(j + 1) * HSL]).astype(np_mm)
           for j in range(NJ)]

    key = (C, MM_DTYPE)
    if key not in _NC_CACHE:
        _NC_CACHE[key] = _build_nc(C)
    nc = _NC_CACHE[key]

    in_maps = []
    for core in range(N_CORES):
        h, j = core // NJ, core % NJ
        in_maps.append({
            "xab": np.ascontiguousarray(xabT[h]),
            "w": wsl[j],
        })

    res = run_bass_kernel_spmd(nc, in_maps, list(range(N_CORES)))
    LAST_RESULTS = res

    y = np.empty((ntok, H), dtype=np.float32)
    for core in range(N_CORES):
        h, j = core // NJ, core % NJ
        toks, rows = assemble[h]
        y[toks, j * HSL:(j + 1) * HSL] = res.results[core]["y"][rows, :]
    return y
